# revision 1
# baseline (speedup 1.0000x reference)
"""Causal self-attention (B=4, S=2048, E=1024, H=16, hd=64) on 8 TRN2 NeuronCores.

Sharding: tensor-parallel over (batch, head-half). Core c handles batch c%4 and
heads [8*(c//4), 8*(c//4)+8) -- i.e. a 512-wide slice of the Wq/Wk/Wv columns
and of the Wo rows. Each core computes a partial [S, E] c_proj output; the host
sums the two half partials per batch and adds bo (the "all-reduce" is a
host-side add of 8 x 32MB partials, which is trivial next to kernel time).

Per-core kernel structure (one batch, 8 heads, everything in transposed form):
  x[S,E] --PE-transpose--> xT[E,S]
  qT = (Wq_sl^T @ xT) * (1/8) + bq_sl*(1/8)   [512, S]   (scale folded in)
  kT = Wk_sl^T @ xT + bk_sl                   [512, S]
  vT = Wv_sl^T @ xT + bv_sl --PE-transpose--> v[S, 64+1] per head (ones col)
  per head, per 512-wide query chunk ic, over key tiles jt (causal):
     S_T[j, i] = kT_h[:, jt]^T-form matmul qT_h[:, ic]   (scores transposed)
     P_T = exp(S_T)            (no max-subtraction needed: |scores| <~ 7)
     causal mask on diagonal tiles via gpsimd affine_select (fill 0)
     yT_aug[65, ic] += v_aug_h[jt]^T-form matmul P_T     (row 64 = softmax sums)
  yT = yT_aug[0:64] / yT_aug[64]  (reciprocal + partition-broadcast DMA + mul)
  out_partial = yT_core^T-form matmul Wo_sl   [S, E]   (bo added on host)

Matmuls run as float32r (full PE rate at N>=256; fp32 data path with relaxed
multiply precision). PE transposes stay fp32 (exact).
"""

import numpy as np

import concourse.bass as bass
from concourse import bacc
import concourse.mybir as mybir
import concourse.tile as tile
from concourse.bass_utils import run_bass_kernel_spmd
from concourse.masks import make_identity

# Problem dims (hardcoded per contract)
B, S, E, H, HD = 4, 2048, 1024, 16, 64
NCORES = 8
EH = 512            # per-core slice of E (8 heads)
NHP = 4             # head pairs per core (2 heads share a 128-partition tile)
NPT = EH // 128     # 4 partition tiles of the per-core head slice
NKT = E // 128      # 8 contraction tiles over E
NST = S // 128      # 16 s-tiles
NIC = S // 512      # 4 query chunks
SCALE = 1.0 / np.sqrt(HD)

F32 = mybir.dt.float32
F32R = mybir.dt.float32r
BF16 = mybir.dt.bfloat16

_CACHED_NC = {}


def _mm(ap, mode):
    """Bitcast an AP to the matmul compute dtype (no-op if already typed)."""
    if mode == "fp32r" and ap.dtype != F32R:
        return ap.bitcast(F32R)
    return ap


def build_bass(mode="fp32r"):
    """Build the single-core SPMD Bass program (same program on all 8 cores)."""
    nc = bacc.Bacc()
    x_h = nc.declare_dram_parameter("x", [S, E], F32, isOutput=False)
    wq_h = nc.declare_dram_parameter("wq", [E, EH], F32, isOutput=False)
    wk_h = nc.declare_dram_parameter("wk", [E, EH], F32, isOutput=False)
    wv_h = nc.declare_dram_parameter("wv", [E, EH], F32, isOutput=False)
    wo_h = nc.declare_dram_parameter("wo", [EH, E], F32, isOutput=False)
    bq_h = nc.declare_dram_parameter("bq", [EH], F32, isOutput=False)  # pre-scaled by 1/8
    bk_h = nc.declare_dram_parameter("bk", [EH], F32, isOutput=False)
    bv_h = nc.declare_dram_parameter("bv", [EH], F32, isOutput=False)
    out_h = nc.declare_dram_parameter("out", [S, E], F32, isOutput=True)

    with tile.TileContext(nc) as tc:
        _build_body(nc, tc, x_h, wq_h, wk_h, wv_h, wo_h, bq_h, bk_h, bv_h, out_h, mode)
    if not nc.is_finalized():
        nc.finalize()
    return nc


def _build_body(nc, tc, x_h, wq_h, wk_h, wv_h, wo_h, bq_h, bk_h, bv_h, out_h, mode):
    import contextlib

    MD = F32 if mode == "fp32" else F32R   # stationary (lhsT) tile dtype
    MV = BF16 if mode == "mixed" else MD   # moving (rhs) tile dtype

    Exp = mybir.ActivationFunctionType.Exp
    Copy = mybir.ActivationFunctionType.Copy
    Ident = mybir.ActivationFunctionType.Identity

    with contextlib.ExitStack() as ctx:
        const = ctx.enter_context(tc.tile_pool(name="const", bufs=1))
        big = ctx.enter_context(tc.tile_pool(name="big", bufs=1))
        ps_grp = ctx.enter_context(tc.tile_pool(name="ps_grp", bufs=3, space="PSUM"))
        ps_y = ctx.enter_context(tc.tile_pool(name="ps_y", bufs=2, space="PSUM"))

        identity = const.tile([128, 128], F32, name="identity")
        make_identity(nc, identity[:])
        # f32 ones staging tile (memset can't write float32r/bf16-typed v2 col)
        ones_f32 = const.tile([128, 64], F32, name="ones_f32")
        nc.gpsimd.memset(ones_f32[:], 1.0)
        # ones row used as K=1 lhsT to broadcast the softmax-sum row across
        # 64 psum partitions (row 64 so base_partition matches the sums row)
        ones_col = const.tile([128, 64], MD, name="ones_col")
        nc.vector.tensor_copy(ones_col[:], ones_f32[:])
        # upper-triangular (keep y >= p) multiplicative mask for the 128-wide
        # diagonal strip of each causal block
        mdt = F32 if MV == F32R else MV
        trimask = const.tile([128, 128], mdt, name="trimask")
        nc.gpsimd.memset(trimask[:], 1.0)
        nc.gpsimd.affine_select(
            out=trimask[:],
            in_=trimask[:],
            compare_op=mybir.AluOpType.is_ge,
            fill=0.0,
            base=0,
            pattern=[[1, 128]],
            channel_multiplier=-1,
        )
        bq_sb = const.tile([128, NPT], F32, name="bq_sb")
        bk_sb = const.tile([128, NPT], F32, name="bk_sb")
        bv_sb = const.tile([128, NPT], F32, name="bv_sb")
        nc.sync.dma_start(bq_sb[:], bq_h[:].rearrange("(o p) -> p o", p=128))
        nc.sync.dma_start(bk_sb[:], bk_h[:].rearrange("(o p) -> p o", p=128))
        nc.sync.dma_start(bv_sb[:], bv_h[:].rearrange("(o p) -> p o", p=128))

        # Long-lived activation tensors
        qT = big.tile([128, NHP, S], MV, name="qT")   # [d(2 heads), hp, s]
        kT = big.tile([128, NHP, S], MV, name="kT")
        # v2[hp]: [j-in-tile, jt, head-in-pair, hd+1]; col 64 = ones (softmax sums)
        v2 = [big.tile([128, NST, 2, HD + 1], MV, name=f"v2_{hp}") for hp in range(NHP)]
        for hp in range(NHP):
            nc.vector.tensor_copy(
                v2[hp][:, :, :, HD : HD + 1],
                ones_f32[:, 0 : 2 * NST].rearrange(
                    "p (a b c) -> p a b c", a=NST, b=2
                ),
            )

        # ---------------- Phase A+B: x transpose + QKV projections --------------
        with tc.tile_pool(name="xphase", bufs=1) as xph, \
             tc.tile_pool(name="xs", bufs=5) as xs_pool, \
             tc.tile_pool(name="wstream", bufs=2) as w_pool, \
             tc.tile_pool(name="vtmp", bufs=2) as vt_pool:
            xT = xph.tile([128, NKT, S], MD, name="xT")  # [e-in-tile, kt, s]

            _sc = nc.enter_named_scope("xpose", False)[0]
            for st in range(NST):
                x_t = xs_pool.tile([128, E], F32, tag="xs")
                nc.sync.dma_start(x_t[:], x_h[:][st * 128 : (st + 1) * 128, :])
                for kg in range(2):
                    ps2t = ps_grp.tile([128, 2, 512], F32, tag="grp", name="ps2t")
                    ps = ps2t[:, 0, :]
                    for j in range(4):
                        kt = kg * 4 + j
                        nc.tensor.transpose(
                            ps[:, j * 128 : (j + 1) * 128],
                            x_t[:, kt * 128 : (kt + 1) * 128],
                            identity[:],
                        )
                    # psum group -> xT[:, kg*4:(kg+1)*4, st*128:(st+1)*128]
                    nc.scalar.activation(
                        xT[:, kg * 4 : (kg + 1) * 4, st * 128 : (st + 1) * 128],
                        ps[:].rearrange("p (a b) -> p a b", a=4),
                        Copy,
                    )

            nc.leave_named_scope("xpose", _sc, False)
            _sc = nc.enter_named_scope("proj", False)[0]
            # projections: loop proj -> pt -> sc; weights streamed per (proj, pt)
            projs = [
                ("q", wq_h, bq_sb, SCALE, qT),
                ("k", wk_h, bk_sb, 1.0, kT),
                ("v", wv_h, bv_sb, 1.0, None),
            ]
            for pname, w_h, b_sb, p_scale, outT in projs:
                w_r = w_h[:].rearrange("(ko p) m -> p ko m", p=128)
                for pt in range(NPT):
                    wt = w_pool.tile([128, NKT, 128], MD, tag="w")
                    nc.sync.dma_start(wt[:], w_r[:, :, pt * 128 : (pt + 1) * 128].bitcast(MD))
                    for sc in range(NIC):
                        acc2 = ps_grp.tile([128, 2, 512], F32, tag="grp", name="acc2")
                        acc = acc2[:, 0, :]
                        for kt in range(NKT):
                            nc.tensor.matmul(
                                acc[:],
                                lhsT=_mm(wt[:, kt, :], mode),
                                rhs=_mm(xT[:, kt, sc * 512 : (sc + 1) * 512], mode),
                                start=(kt == 0),
                                stop=(kt == NKT - 1),
                            )
                        if outT is not None:
                            nc.scalar.activation(
                                outT[:, pt, sc * 512 : (sc + 1) * 512],
                                acc[:],
                                Ident,
                                bias=b_sb[:, pt : pt + 1],
                                scale=p_scale,
                            )
                        else:
                            # v path: bias-copy to temp, PE-transpose into v2[pt]
                            vtmp = vt_pool.tile([128, 512], F32, tag="vt")
                            nc.scalar.activation(
                                vtmp[:], acc[:], Ident, bias=b_sb[:, pt : pt + 1]
                            )
                            ps2w = ps_grp.tile([128, 2, 512], F32, tag="grp", name="ps2w")
                            ps2 = ps2w[:, 0, :]
                            for j in range(4):
                                nc.tensor.transpose(
                                    ps2[:, j * 128 : (j + 1) * 128],
                                    vtmp[:, j * 128 : (j + 1) * 128],
                                    identity[:],
                                )
                            # ps2: [s-in-block, (block j, head hh, d 64x2)]
                            nc.vector.tensor_copy(
                                v2[pt][:, sc * 4 : (sc + 1) * 4, :, 0:HD],
                                ps2[:].rearrange("p (a b c) -> p a b c", a=4, b=2),
                            )

            nc.leave_named_scope("proj", _sc, False)

        # ---------------- Phase C+D: attention + output projection ----------------
        with tc.tile_pool(name="att", bufs=1) as att_pool, \
             tc.tile_pool(name="ptp", bufs=6) as pt_pool, \
             tc.tile_pool(name="rcp", bufs=4) as rc_pool, \
             tc.tile_pool(name="wop", bufs=1) as wo_pool, \
             tc.tile_pool(name="outp", bufs=3) as out_pool:
            yT = att_pool.tile([128, NPT, S], MD, name="yT")
            wo_sb = wo_pool.tile([128, NPT, E], MD, name="wo_sb")
            nc.sync.dma_start(
                wo_sb[:], wo_h[:].rearrange("(ko p) n -> p ko n", p=128).bitcast(MD)
            )

            for ic in range(NIC):
                _sc = nc.enter_named_scope(f"attn{ic}", False)[0]
                for hp in range(NHP):
                    njt = 4 * ic + 4
                    psy = [
                        ps_y.tile([128, 512], F32, tag="y", name=f"psy{i}")
                        for i in range(2)
                    ]
                    stage = []  # pending (jt, [(ps_s, pt_t, hh)...]) awaiting exp+PV

                    def flush(jt_p, ps_s, pt_t, njt=njt, ic=ic, hp=hp, psy=psy):
                        r = jt_p - 4 * ic
                        w0 = 128 * r if r >= 0 else 0
                        W = 512 - w0
                        # one wide exp covers both heads' score halves
                        nc.scalar.activation(
                            pt_t[:, :, w0:512],
                            ps_s[:, :, 0:W],
                            Exp,
                        )
                        if r >= 0:
                            # only the first 128 cols of the window are
                            # partially masked (j <= 127 < i elsewhere)
                            for hh in range(2):
                                nc.vector.tensor_mul(
                                    pt_t[:, hh, w0 : w0 + 128],
                                    pt_t[:, hh, w0 : w0 + 128],
                                    trimask[:],
                                )
                        for hh in range(2):
                            nc.tensor.matmul(
                                psy[hh][0:HD + 1, w0:512],
                                lhsT=_mm(v2[hp][:, jt_p, hh, :], mode),
                                rhs=_mm(pt_t[:, hh, w0:512], mode),
                                start=(jt_p == 0),
                                stop=(jt_p == njt - 1),
                            )

                    for jt in range(njt):
                        r = jt - 4 * ic
                        w0 = 128 * r if r >= 0 else 0
                        W = 512 - w0
                        ps_s = ps_grp.tile([128, 2, 512], F32, tag="grp")
                        for hh in range(2):
                            base = hh * 64
                            nc.tensor.matmul(
                                ps_s[:, hh, 0:W],
                                lhsT=_mm(
                                    kT[base : base + 64, hp, jt * 128 : (jt + 1) * 128],
                                    mode,
                                ),
                                rhs=_mm(
                                    qT[base : base + 64, hp, ic * 512 + w0 : (ic + 1) * 512],
                                    mode,
                                ),
                                start=True,
                                stop=True,
                            )
                        pt_t = pt_pool.tile([128, 2, 512], MV, tag="pt")
                        stage.append((jt, ps_s, pt_t))
                        if len(stage) > 2:
                            flush(*stage.pop(0))
                    while stage:
                        flush(*stage.pop(0))

                    # normalize: yT = psy[0:64] * (1 / psy[64])
                    for hh in range(2):
                        srow = rc_pool.tile([128, 512], MD, tag="srow")
                        # sums row to SBUF (lane 64 -> lane 64)
                        nc.vector.tensor_copy(srow[64:65, :], psy[hh][64:65, :])
                        # K=1 matmul broadcasts the sums row across 64 psum
                        # partitions; fast DVE reciprocal lands it in SBUF
                        bc_ps2 = ps_grp.tile([128, 2, 512], F32, tag="grp", name="bc_ps2")
                        bc_ps = bc_ps2[:, 0, :]
                        nc.tensor.matmul(
                            bc_ps[0:64, :],
                            lhsT=ones_col[64:65, :],
                            rhs=srow[64:65, :],
                            start=True,
                            stop=True,
                        )
                        rrow = rc_pool.tile([64, 512], F32, tag="rrow")
                        nc.vector.reciprocal_approx_fast(
                            rrow[:, :], bc_ps[0:64, :]
                        )
                        nc.vector.tensor_mul(
                            yT[hh * 64 : hh * 64 + 64, hp, ic * 512 : (ic + 1) * 512],
                            psy[hh][0:64, :],
                            rrow[:, :],
                        )

                # c_proj for the 4 s-tiles of this query chunk
                for st in range(4 * ic, 4 * ic + 4):
                    ot = out_pool.tile([128, E], F32, tag="ot")
                    for ec in range(2):
                        acc2 = ps_grp.tile([128, 2, 512], F32, tag="grp", name="acc2c")
                        acc = acc2[:, 0, :]
                        for ptd in range(NPT):
                            nc.tensor.matmul(
                                acc[:],
                                lhsT=_mm(yT[:, ptd, st * 128 : (st + 1) * 128], mode),
                                rhs=_mm(wo_sb[:, ptd, ec * 512 : (ec + 1) * 512], mode),
                                start=(ptd == 0),
                                stop=(ptd == NPT - 1),
                            )
                        nc.vector.tensor_copy(ot[:, ec * 512 : (ec + 1) * 512], acc[:])
                    nc.sync.dma_start(out_h[:][st * 128 : (st + 1) * 128, :], ot[:])
                nc.leave_named_scope(f"attn{ic}", _sc, False)


def _get_nc(mode="mixed"):
    if mode not in _CACHED_NC:
        _CACHED_NC[mode] = build_bass(mode)
    return _CACHED_NC[mode]


def make_in_maps(x, Wq, bq, Wk, bk, Wv, bv, Wo, bo):
    in_maps = []
    for c in range(NCORES):
        b = c % B
        half = c // B
        sl = slice(half * EH, (half + 1) * EH)
        in_maps.append(
            {
                "x": np.ascontiguousarray(x[b]),
                "wq": np.ascontiguousarray(Wq[:, sl]),
                "wk": np.ascontiguousarray(Wk[:, sl]),
                "wv": np.ascontiguousarray(Wv[:, sl]),
                "wo": np.ascontiguousarray(Wo[sl, :]),
                "bq": np.ascontiguousarray(bq[sl]) * np.float32(SCALE),
                "bk": np.ascontiguousarray(bk[sl]),
                "bv": np.ascontiguousarray(bv[sl]),
            }
        )
    return in_maps


def assemble(results, bo):
    out = np.empty((B, S, E), dtype=np.float32)
    for b in range(B):
        out[b] = results[b]["out"] + results[b + B]["out"] + bo[None, :]
    return out


def kernel(x, Wq, bq, Wk, bk, Wv, bv, Wo, bo, _trace=False, _mode="mixed"):
    x = np.asarray(x, dtype=np.float32)
    Wq = np.asarray(Wq, dtype=np.float32)
    bq = np.asarray(bq, dtype=np.float32)
    Wk = np.asarray(Wk, dtype=np.float32)
    bk = np.asarray(bk, dtype=np.float32)
    Wv = np.asarray(Wv, dtype=np.float32)
    bv = np.asarray(bv, dtype=np.float32)
    Wo = np.asarray(Wo, dtype=np.float32)
    bo = np.asarray(bo, dtype=np.float32)

    nc = _get_nc(_mode)
    in_maps = make_in_maps(x, Wq, bq, Wk, bk, Wv, bv, Wo, bo)
    res = run_bass_kernel_spmd(nc, in_maps, list(range(NCORES)), trace=_trace)
    out = assemble(res.results, bo)
    if _trace:
        return out, res
    return out

